# revision 40
# baseline (speedup 1.0000x reference)
"""AdaptiveQuadratureHead Trainium2 kernel.

8-core SPMD, data-parallel over batch (B=8 -> one batch element per core).
No collectives. Host marshals inputs into feature-major bf16 layouts with a
virtual token order (physical token q*64+t -> tile t, partition q) so that
per-token scalars (quadrature weights) land as contiguous [128, 1] columns,
and packs all weights/constants into four DMA-friendly blocks.

Per core (N=8192 tokens), all matmuls bf16 with fp32 PSUM accumulation:
  pass A (per 512-token chunk, ACT-saturated):
    K = gelu(gelu(xu @ W1k + kb1) @ kW2 + kb2) @ kW3 + kb3  (feature-major;
        even/odd chunks land in partitions 0-63/64-127 via PE col-tiling)
    V = gelu(gelu(xu @ W1v + vb1) @ vW2 + vb2) @ vW3        (token-major via
        h2-stationary matmuls); Vw = V*w accumulated for c
    w = max(sensor_weights, 0) * mask ; denom = max(sum w, eps)
  pass B (serial, tiny): c = (sum w*V)/denom + vb3 ;
    a2d = tanh(gelu(c@aW1+ab1) @ aW2perm + ab2) via per-r column matmuls,
    PE-transposed to aT ; Q^T = qT + (0.1*B_dirs)^T a^T
  pass C (per 8-tile group): scoresT = K^T Q with two concurrent row-group
    matmuls; softplus(s/8) ~ ln2 + s/16 + s^2/512 = (s+16)^2/512 + (ln2-1/2)
    -> ONE ACT Square op; the constant folds into pooled via sum(w*V);
    exact ln(exp(s/8)+1) fallback when a sampled score bound is large.
    pooledT accumulated as (Vw | w)^T @ Phi matmuls.
  pass D: pooledT/denom (+vb3, +ln2 corrections), rho MLP, out [128, 128].
"""

import os
import sys
import types
from contextlib import ExitStack

import numpy as np
import ml_dtypes

B, N = 8, 8192
DX, DU = 64, 4
P, DK, DV, DOUT = 128, 64, 64, 128
H, R, AH = 256, 4, 64
ADAPT_SCALE = 0.1
EPS = 1e-8
NT = 64          # token tiles of 128 (virtual order)
NCH = 16         # column chunks of 512
VW_W = 65        # V*w tile width (64 features + w column)
NG = 8           # score groups (8 tiles of 128 -> [128, 1024])
BF16 = ml_dtypes.bfloat16

# bf16 weight pack column offsets (rows 0-127)
WB_COLS = {
    "kW1": (0, 256), "vW1": (256, 512),
    "kW2a": (512, 768), "kW2b": (768, 1024),
    "vW2a": (1024, 1280), "vW2b": (1280, 1536),
    "kW3a": (1536, 1600), "kW3b": (1600, 1664),
    "vW3a": (1664, 1728), "vW3b": (1728, 1792),
    "rW2a": (1792, 1920), "rW2b": (1920, 2048),
    "rW1": (2048, 2304), "rb2": (2304, 2432), "BdTf": (2432, 2688),
}
WB_W = 2688
# f32 pack A [128, 138]
FA_COLS = {"sw": (0, 64), "mk": (64, 128), "kb1": (128, 130),
           "vb1": (130, 132), "kb2": (132, 134), "vb2": (134, 136),
           "rb1": (136, 138), "qT128": (138, 266), "kb3b": (266, 267),
           "ab2d": (267, 271)}
FA_W = 271
# f32 pack B [64, 1283]
FB_COLS = {"qT": (0, 128), "aW2": (128, 640), "aW1": (640, 704),
           "ab1": (704, 705), "kb3": (705, 706), "vb3": (706, 707),
           "Bd": (707, 771), "ab2": (771, 1283)}
FB_W = 1283

_CACHE = {}


def _install_ntff_hook_shim():
    """This image's antenv lacks axon_hooks; provide it so
    run_bass_kernel_spmd(trace=True) can reach the ctypes NTFF hook."""
    if "antenv.axon_hooks" in sys.modules:
        return
    try:
        from trn_agent_boot.trn_boot import _ntff_profile_via_ctypes
        hook = _ntff_profile_via_ctypes("/opt/axon/libaxon_pjrt.so")
    except Exception:
        hook = None
    mod = types.ModuleType("antenv.axon_hooks")
    mod._hook = hook
    mod.get_axon_ntff_profile_hook = lambda: mod._hook
    mod.set_axon_ntff_profile_hook = lambda h: setattr(mod, "_hook", h)
    sys.modules["antenv.axon_hooks"] = mod
    try:
        import antenv
        antenv.axon_hooks = mod
    except Exception:
        pass


def _build(biases_zero: bool, vb3_zero: bool, ab2_zero: bool, rb2_zero: bool,
           use_poly: bool, rb1_zero: bool, use_fp8: bool):
    import concourse.bass as bass
    import concourse.bacc as bacc
    import concourse.mybir as mybir
    import concourse.tile as tile
    from concourse import masks
    from concourse.tile_rust import add_dep_helper

    AF = mybir.ActivationFunctionType
    OP = mybir.AluOpType
    AX = mybir.AxisListType
    f32 = mybir.dt.float32
    bf16 = mybir.dt.bfloat16
    f8 = mybir.dt.float8e4

    nc = bacc.Bacc(None, target_bir_lowering=False)

    def din(name, shape, dt):
        return nc.declare_dram_parameter(name, list(shape), dt, isOutput=False)

    xu_d = din("xu_fm", (68, N), bf16)
    wblo_d = din("wb16lo", (68, 512), bf16)
    wbhi_d = din("wb16hi", (128, WB_W - 512), bf16)
    fa_d = din("f32a", (128, FA_W), f32)
    fb_d = din("f32b", (64, FB_W), f32)
    w8_d = din("w8", (128, 1024), f8) if use_fp8 else None
    out_d = nc.declare_dram_parameter("out", [P, DOUT], f32, isOutput=True)

    with tile.TileContext(nc) as tc, ExitStack() as ctx:
        const = ctx.enter_context(tc.tile_pool(name="const", bufs=1))
        xu_pool = ctx.enter_context(tc.tile_pool(name="xu", bufs=5))
        h_pool = ctx.enter_context(tc.tile_pool(name="hsb", bufs=6))
        phi_pool = ctx.enter_context(tc.tile_pool(name="phi", bufs=4))
        es_pool = (None if use_poly else
                   ctx.enter_context(tc.tile_pool(name="es", bufs=NG)))
        ps_big = ctx.enter_context(
            tc.tile_pool(name="psb", bufs=3, space=bass.MemorySpace.PSUM))
        ps_sm = ctx.enter_context(
            tc.tile_pool(name="pss", bufs=2, space=bass.MemorySpace.PSUM))

        # ---- packed constants. l1 weights are a SEPARATE small tile and
        # the first DMA issued, so the first matmul is not serialized
        # behind the big wbhi transfer (tile-granular write tracking). ----
        wblo_t = const.tile([68, 512], bf16, tag="wblo")
        nc.gpsimd.dma_start(wblo_t[:], wblo_d[:])
        wbhi_t = const.tile([128, WB_W - 512], bf16, tag="wbhi")
        nc.gpsimd.dma_start(wbhi_t[:], wbhi_d[:])
        fa_t = const.tile([128, FA_W], f32, tag="fa")
        nc.gpsimd.dma_start(fa_t[:], fa_d[:])
        fb_t = const.tile([64, FB_W], f32, tag="fb")
        nc.gpsimd.dma_start(fb_t[:], fb_d[:])
        if use_fp8:
            w8_t = const.tile([128, 1024], f8, tag="w8")
            nc.gpsimd.dma_start(w8_t[:], w8_d[:])

        def wb(name, rows=128):
            c0, c1 = WB_COLS[name]
            if c1 <= 512:
                return wblo_t[0:rows, c0:c1]
            return wbhi_t[0:rows, c0 - 512:c1 - 512]

        def fa(name):
            c0, c1 = FA_COLS[name]
            return fa_t[:, c0:c1]

        def fb(name, rows=64):
            c0, c1 = FB_COLS[name]
            return fb_t[0:rows, c0:c1]

        K_fm2 = const.tile([128, N // 2], bf16, tag="K_fm")
        Vw_all = const.tile([128, NT * VW_W], bf16, tag="Vw")
        Vacc = const.tile([128, 4 * DV], f32, tag="Vacc")
        Vacc2 = const.tile([128, 4 * DV], f32, tag="Vacc2")
        w_t = const.tile([128, NT], f32, tag="w")
        ones_col = const.tile([128, 1], f32, tag="ones_c")
        ones_r64 = const.tile([1, 64], f32, tag="ones_r")
        ones_rP = const.tile([1, P], bf16, tag="ones_p")
        sqb_t = const.tile([128, 1], f32, tag="sqb")
        nc.vector.memset(sqb_t[:], float(16.0 / np.sqrt(512.0)))
        wsum = const.tile([128, 1], f32, tag="wsum")
        den_t = const.tile([1, 1], f32, tag="den")
        rec_t = const.tile([1, 1], f32, tag="rec")
        recb_t = const.tile([64, 1], f32, tag="recb")

        nc.vector.memset(Vacc[:], 0.0)
        nc.vector.memset(Vacc2[:], 0.0)
        nc.vector.memset(ones_col[:], 1.0)
        nc.vector.memset(ones_r64[:], 1.0)
        nc.vector.memset(ones_rP[:], 1.0)

        # quadrature weights: w = max(sw, 0) * mask
        nc.vector.tensor_scalar(w_t[:], fa("sw"), 0.0, None, OP.max)
        nc.vector.tensor_tensor(w_t[:], w_t[:], fa("mk"), op=OP.mult)
        nc.vector.reduce_sum(wsum[:], w_t[:], axis=AX.X)

        Vw_v = Vw_all[:].rearrange("p (t c) -> p t c", c=VW_W)

        # ================= PASS A: token MLPs (K+V fused) =================
        # K-side of the last TAIL chunks is deferred until after pass B so
        # the serial c->Q chain hides under real PE/ACT work; score groups
        # for earlier chunks don't depend on them.
        TAIL = 2
        xu_tiles = {}
        h1dt = f8 if use_fp8 else bf16

        def l2_mm(h2_ps, h1, w8lo, wa, wbname, fp8_ok=False):
            if use_fp8 and fp8_ok:
                h1_v = h1[:].rearrange("p (ko n) -> p ko n", ko=2)
                dr = w8_t[:, w8lo:w8lo + 512].rearrange(
                    "p (ko m) -> p ko m", ko=2)
                for ho in range(2):
                    hs = slice(ho * 512, (ho + 1) * 512)
                    wsl = slice(ho * 128, (ho + 1) * 128)
                    nc.tensor.matmul(
                        h2_ps[:, hs], dr[:, :, wsl], h1_v,
                        perf_mode=mybir.MatmulPerfMode.DoubleRow)
            else:
                for ho in range(2):
                    hs = slice(ho * 512, (ho + 1) * 512)
                    wsl = slice(ho * 128, (ho + 1) * 128)
                    nc.tensor.matmul(h2_ps[:, hs], wb(wa)[:, wsl],
                                     h1[:, 0:512], start=True, stop=False)
                    nc.tensor.matmul(h2_ps[:, hs], wb(wbname)[:, wsl],
                                     h1[:, 512:1024], start=False, stop=True)

        def gelu2(dst, src_ps, bcol):
            if biases_zero:
                nc.scalar.activation(dst[:], src_ps[:], AF.Gelu)
            else:
                for ho in range(2):
                    hs = slice(ho * 512, (ho + 1) * 512)
                    nc.scalar.activation(dst[:, hs], src_ps[:, hs], AF.Gelu,
                                         bias=fa(bcol)[:, ho:ho + 1])

        def k_tail(j, h2K):
            # L3 K-net; K lands in partitions 0-63 (even chunks) / 64-127
            # (odd) so score matmuls later run two row-groups concurrently.
            kr = slice(0, 64) if j % 2 == 0 else slice(64, 128)
            kcs = slice((j // 2) * 512, (j // 2 + 1) * 512)
            k_ps = ps_sm.tile([128, 512], f32, tag="ps")
            tp = (0, 0) if j % 2 == 0 else (0, 64)
            nc.tensor.matmul(k_ps[kr, :], wb("kW3a"), h2K[:, 0:512],
                             start=True, stop=False, tile_position=tp)
            nc.tensor.matmul(k_ps[kr, :], wb("kW3b"), h2K[:, 512:1024],
                             start=False, stop=True, tile_position=tp)
            nc.vector.tensor_scalar_add(K_fm2[kr, kcs], k_ps[kr, :],
                                        fa("kb3b")[kr, :])

        pending = {}

        def tail_work(j):
            # L3 of chunk j, deferred into chunk j+1's l1->l2 window so the
            # PE never stalls waiting for gelu(h1K) of the current chunk.
            if j not in pending:
                return
            h2K, h2V = pending.pop(j)
            if h2K is not None:
                k_tail(j, h2K)
                xu_tiles.pop(j)
            # L3 V-net: token-major V tiles [128tok, 64]
            v_ps = ps_sm.tile([128, 4 * DV], f32, tag="ps")
            for st in range(4):
                vs = slice(st * DV, (st + 1) * DV)
                ts_a = slice(st * 128, (st + 1) * 128)
                ts_b = slice(512 + st * 128, 512 + (st + 1) * 128)
                nc.tensor.matmul(v_ps[:, vs], h2V[:, ts_a], wb("vW3a"),
                                 start=True, stop=False)
                nc.tensor.matmul(v_ps[:, vs], h2V[:, ts_b], wb("vW3b"),
                                 start=False, stop=True)

            tsl = slice(4 * j, 4 * j + 4)
            w_b = w_t[:, tsl].to_broadcast([128, 4, DV])
            v4 = v_ps[:].rearrange("p (t c) -> p t c", c=DV)
            nc.vector.tensor_tensor(Vw_v[:, tsl, 0:DV], v4, w_b, op=OP.mult)
            if not vb3_zero:
                nc.vector.tensor_copy(
                    Vw_v[:, tsl, DV:VW_W],
                    w_t[:, tsl].rearrange("p (t c) -> p t c", c=1))
            vac = Vacc if j % 2 == 0 else Vacc2
            nc.vector.tensor_tensor(vac[:], vac[:], Vw_v[:, tsl, 0:DV],
                                    op=OP.add)

        def body(j, do_k):
            cs = slice(j * 512, (j + 1) * 512)
            xu = xu_pool.tile([68, 512], bf16, tag="xu")
            nc.sync.dma_start(xu[:], xu_d[:, cs])
            xu_tiles[j] = xu

            if do_k:
                h1K_ps = ps_big.tile([128, 1024], f32, tag="ps")
                nc.tensor.matmul(h1K_ps[:, 0:512], wb("kW1", 68)[:, 0:128],
                                 xu[:])
                nc.tensor.matmul(h1K_ps[:, 512:1024],
                                 wb("kW1", 68)[:, 128:256], xu[:])
            h1V_ps = ps_big.tile([128, 1024], f32, tag="ps")
            nc.tensor.matmul(h1V_ps[:, 0:512], wb("vW1", 68)[:, 0:128], xu[:])
            nc.tensor.matmul(h1V_ps[:, 512:1024], wb("vW1", 68)[:, 128:256],
                             xu[:])

            tail_work(j - 1)

            # denom = max(sum w, eps): PE hops placed in successive tail
            # slots so the DVE round-trips never stall the PE queue
            if j == 3:
                den_ps = ps_sm.tile([1, 1], f32, tag="ps")
                nc.tensor.matmul(den_ps[:], wsum[:], ones_col[:])
                nc.vector.tensor_scalar(den_t[:], den_ps[:], EPS, None,
                                        OP.max)
                nc.vector.reciprocal(rec_t[:], den_t[:])
            elif j == 4:
                recb_ps = ps_sm.tile([64, 1], f32, tag="ps")
                nc.tensor.matmul(recb_ps[:], ones_r64[:], rec_t[:])
                nc.vector.tensor_copy(recb_t[:], recb_ps[:])

            if do_k:
                h1K = h_pool.tile([128, 1024], bf16, tag="h")
                gelu2(h1K, h1K_ps, "kb1")
            h1V = h_pool.tile([128, 1024], h1dt, tag="h")
            gelu2(h1V, h1V_ps, "vb1")

            if do_k:
                h2K_ps = ps_big.tile([128, 1024], f32, tag="ps")
                l2_mm(h2K_ps, h1K, 0, "kW2a", "kW2b")
            h2V_ps = ps_big.tile([128, 1024], f32, tag="ps")
            l2_mm(h2V_ps, h1V, 512, "vW2a", "vW2b", fp8_ok=True)

            if do_k:
                h2K = h_pool.tile([128, 1024], bf16, tag="h")
                gelu2(h2K, h2K_ps, "kb2")
            h2V = h_pool.tile([128, 1024], bf16, tag="h")
            gelu2(h2V, h2V_ps, "vb2")
            pending[j] = (h2K if do_k else None, h2V)

        def k_part_l1(j):
            xu = xu_tiles.pop(j)
            h1K_ps = ps_big.tile([128, 1024], f32, tag="ps")
            nc.tensor.matmul(h1K_ps[:, 0:512], wb("kW1", 68)[:, 0:128], xu[:])
            nc.tensor.matmul(h1K_ps[:, 512:1024], wb("kW1", 68)[:, 128:256],
                             xu[:])
            h1K = h_pool.tile([128, 1024], bf16, tag="h")
            gelu2(h1K, h1K_ps, "kb1")
            return h1K

        def k_part_l2(j, h1K):
            h2K_ps = ps_big.tile([128, 1024], f32, tag="ps")
            l2_mm(h2K_ps, h1K, 0, "kW2a", "kW2b")
            h2K = h_pool.tile([128, 1024], bf16, tag="h")
            gelu2(h2K, h2K_ps, "kb2")
            k_tail(j, h2K)

        # V-only chunks spread mid-stream (not back-to-back at the end) so
        # neighboring full chunks' K gelus fill the ACT holes they leave
        DEFER = (12, 14)
        for j in range(NCH):
            body(j, do_k=(j not in DEFER))
        tail_work(NCH - 1)

        # ====== PASS B, interleaved at half-chunk granularity with the
        # deferred K chunks so every hop of the serial c -> Q chain hides
        # under real PE/ACT work ======
        ln2c_t = (const.tile([DV, 1], f32, tag="ln2c", name="ln2c_t")
                  if use_poly else None)
        g_t = const.tile([AH, 1], f32, tag="g")
        a_row = const.tile([1, P * R], bf16, tag="a_row")
        QT_t = const.tile([128, P], bf16, tag="QT")

        def passb_c():
            nc.vector.tensor_tensor(Vacc[:], Vacc[:], Vacc2[:], op=OP.add)
            Vred = const.tile([128, DV], f32, tag="Vred")
            nc.vector.reduce_sum(
                Vred[:], Vacc[:].rearrange("p (t c) -> p c t", c=DV),
                axis=AX.X)
            c_ps = ps_sm.tile([DV, 1], f32, tag="ps")
            nc.tensor.matmul(c_ps[:], Vred[:], ones_col[:])

            # c = c_raw/denom + vb3 (max(denom,eps)=denom when w-sum > eps)
            c_t = const.tile([DV, 1], f32, tag="c")
            nc.vector.scalar_tensor_tensor(c_t[:], c_ps[:], recb_t[:],
                                           fb("vb3"), op0=OP.mult, op1=OP.add)
            if use_poly:
                # pooled uses psi = softplus - ln2; the ln2*sum(wV)/denom
                # correction folds into pass D as a per-feature constant
                nc.vector.tensor_scalar(ln2c_t[:], c_ps[:], recb_t[:],
                                        float(np.log(2.0) - 0.5), OP.mult,
                                        OP.mult)
            # g = gelu(aW1.T c + ab1)
            g_ps = ps_sm.tile([AH, 1], f32, tag="ps")
            nc.tensor.matmul(g_ps[:], fb("aW1"), c_t[:])
            nc.scalar.activation(g_t[:], g_ps[:], AF.Gelu, bias=fb("ab1"))

        def passb_a():
            # a as one row (host pack is r-major): a_row = g.T @ aW2, tanh.
            ar_ps = ps_sm.tile([1, P * R], f32, tag="ps")
            nc.tensor.matmul(ar_ps[:], g_t[:], fb("aW2"),
                             start=True, stop=True)
            if ab2_zero:
                nc.scalar.activation(a_row[:], ar_ps[:], AF.Tanh)
            else:
                a1_t = const.tile([1, P * R], f32, tag="a1")
                nc.vector.tensor_tensor(a1_t[:], ar_ps[:], fb("ab2", 1),
                                        op=OP.add)
                nc.scalar.activation(a_row[:], a1_t[:], AF.Tanh)

        def passb_q():
            # Q^T delta = sum_r outer(Bd[r], a_row[r-block]): four rank-1
            # accumulating matmuls per row-half; no transpose needed.
            qd_ps = ps_sm.tile([128, P], f32, tag="ps")
            bdf = wb("BdTf", 1)
            for r in range(R):
                st = bdf[:, r * DK:(r + 1) * DK]
                mv = a_row[:, r * P:(r + 1) * P]
                nc.tensor.matmul(qd_ps[0:DK, :], st, mv,
                                 start=(r == 0), stop=(r == R - 1))
                nc.tensor.matmul(qd_ps[DK:128, :], st, mv,
                                 start=(r == 0), stop=(r == R - 1),
                                 tile_position=(0, 64))
            nc.vector.tensor_tensor(QT_t[:], qd_ps[:], fa("qT128"), op=OP.add)

        # ================= PASS C machinery =================
        # Phi = softplus(s/sqrt(dk)). With small score range (host-verified),
        # psi = s/16 + s^2/512 = (s+16)^2/512 - 1/2; the constant joins the
        # ln2 pooled-correction, so softplus is ONE ACT Square op.
        # Otherwise: ln(exp(y)+1) on ACT, with all exps batched before all
        # lns because Exp and Ln live in different ACT table sets.
        pool_m = DV if vb3_zero else VW_W
        pool_ps = None  # allocated after pass-B's ps_sm tiles (slot rotation)

        def sc_group(sc_ps, t0, gsz):
            # scoresT per tile: stationary K slice, moving Q^T; even/odd
            # chunks run concurrently in disjoint PE row-halves. (Must
            # stay token-partitioned for the pooled matmuls.)
            for s in range(gsz):
                t = t0 + s
                lo = (t // 4) % 2 == 0
                co = slice((t // 8) * 512 + (t % 4) * 128,
                           (t // 8) * 512 + (t % 4 + 1) * 128)
                if lo:
                    nc.tensor.matmul(sc_ps[:, s * 128:(s + 1) * 128],
                                     K_fm2[0:64, co], QT_t[0:64, :])
                else:
                    nc.tensor.matmul(sc_ps[:, s * 128:(s + 1) * 128],
                                     K_fm2[64:128, co], QT_t[64:128, :],
                                     tile_position=(64, 0))

        GROUPS = [(0, 4), (4, 8), (12, 8), (20, 8), (28, 8), (36, 8),
                  (44, 8), (52, 8), (60, 4)]
        pstate = {"prev": None, "emitted": 0}

        def pooled_mms(phi, t0, gsz):
            for s in range(gsz):
                t = t0 + s
                e = pstate["emitted"]
                pstate["emitted"] += 1
                nc.tensor.matmul(pool_ps[:], Vw_v[:, t, 0:pool_m],
                                 phi[:, s * 128:(s + 1) * 128],
                                 start=(e == 0), stop=(e == NT - 1))

        def emit_group(gi):
            # scores + Square for group gi; pooled of the previous group is
            # emitted after this group's Square (pipelined emission)
            t0, gsz = GROUPS[gi]
            sc_ps = ps_big.tile([128, gsz * 128], f32, tag="ps")
            sc_group(sc_ps, t0, gsz)
            phi = phi_pool.tile([128, gsz * 128], bf16, tag="phi")
            nc.scalar.activation(phi[:], sc_ps[:], AF.Square,
                                 scale=float(1.0 / np.sqrt(512.0)),
                                 bias=sqb_t[:])
            if pstate["prev"] is not None:
                pooled_mms(*pstate["prev"])
            pstate["prev"] = (phi, t0, gsz)

        # ====== endgame: deferred K chunks + pass B chain + pass C groups,
        # interleaved so the ACT queue never starves ======
        h1K_a = k_part_l1(DEFER[0])
        passb_c()
        k_part_l2(DEFER[0], h1K_a)
        passb_a()
        h1K_b = k_part_l1(DEFER[1])
        passb_q()
        pool_ps = ps_sm.tile([pool_m, P], f32, tag="ps", name="pool_ps")

        if use_poly:
            emit_group(0)
            emit_group(1)
            emit_group(2)
            k_part_l2(DEFER[1], h1K_b)
            for gi in (3, 4, 5, 8, 6, 7):
                emit_group(gi)
            pooled_mms(*pstate["prev"])
        else:
            k_part_l2(DEFER[1], h1K_b)
            es_tiles = []
            last_exp = None
            for g in range(NG):
                sc_ps = ps_big.tile([128, 1024], f32, tag="ps")
                sc_group(sc_ps, g * 8, 8)
                es = es_pool.tile([128, 1024], f32, tag="es")
                last_exp = nc.scalar.activation(es[:], sc_ps[:], AF.Exp,
                                                scale=float(1.0 / np.sqrt(DK)))
                es_tiles.append(es)
            for g in range(NG):
                phi = phi_pool.tile([128, 1024], bf16, tag="phi")
                ln_i = nc.scalar.activation(phi[:], es_tiles[g][:], AF.Ln,
                                            bias=1.0)
                add_dep_helper(ln_i.ins, last_exp.ins, sync=False,
                               reason="batch act-table sets")
                for s in range(8):
                    t = g * 8 + s
                    nc.tensor.matmul(pool_ps[:], Vw_v[:, t, 0:pool_m],
                                     phi[:, s * 128:(s + 1) * 128],
                                     start=(t == 0), stop=(t == NT - 1))

        # ================= PASS D: normalize + rho MLP =================
        poolb_t = const.tile([DV, P], bf16, tag="poolb")
        if vb3_zero:
            if use_poly:
                nc.vector.tensor_scalar(poolb_t[:], pool_ps[0:DV, :],
                                        recb_t[:], ln2c_t[:], OP.mult, OP.add)
            else:
                nc.vector.tensor_scalar_mul(poolb_t[:], pool_ps[0:DV, :],
                                            recb_t[:])
        else:
            swp_t = const.tile([1, P], f32, tag="swp")
            if use_poly:
                nc.vector.tensor_scalar(swp_t[:], pool_ps[DV:VW_W, :],
                                        rec_t[:], float(np.log(2.0) - 0.5),
                                        OP.mult, OP.add)
            else:
                nc.vector.tensor_scalar_mul(swp_t[:], pool_ps[DV:VW_W, :],
                                            rec_t[:])
            swpb_ps = ps_sm.tile([DV, P], f32, tag="ps")
            nc.tensor.matmul(swpb_ps[:], ones_r64[:], swp_t[:])
            pooln_t = const.tile([DV, P], f32, tag="pooln")
            if use_poly:
                nc.vector.tensor_scalar(pooln_t[:], pool_ps[0:DV, :],
                                        recb_t[:], ln2c_t[:], OP.mult, OP.add)
            else:
                nc.vector.tensor_scalar_mul(pooln_t[:], pool_ps[0:DV, :],
                                            recb_t[:])
            nc.vector.scalar_tensor_tensor(poolb_t[:], swpb_ps[:], fb("vb3"),
                                           pooln_t[:], op0=OP.mult,
                                           op1=OP.add)

        # rho L1: hr = gelu(rW1.T pooledT + rb1)
        hr_ps = ps_sm.tile([128, 2 * P], f32, tag="ps")
        for hc in range(2):
            nc.tensor.matmul(hr_ps[:, hc * P:(hc + 1) * P],
                             wb("rW1", 64)[:, hc * 128:(hc + 1) * 128],
                             poolb_t[:])
        hr_t = const.tile([128, 2 * P], bf16, tag="hr")
        if rb1_zero:
            nc.scalar.activation(hr_t[:], hr_ps[:], AF.Gelu)
        else:
            for hc in range(2):
                hs = slice(hc * P, (hc + 1) * P)
                nc.scalar.activation(hr_t[:, hs], hr_ps[:, hs], AF.Gelu,
                                     bias=fa("rb1")[:, hc:hc + 1])
        hr_sb = [hr_t[:, 0:P], hr_t[:, P:2 * P]]

        # rho L2: out = hr.T rW2 + rb2  (bias via rank-1 ones matmul)
        o_ps = ps_sm.tile([P, DOUT], f32, tag="ps")
        nc.tensor.matmul(o_ps[:], hr_sb[0], wb("rW2a"),
                         start=True, stop=False)
        nc.tensor.matmul(o_ps[:], hr_sb[1], wb("rW2b"),
                         start=False, stop=rb2_zero)
        if not rb2_zero:
            nc.tensor.matmul(o_ps[:], ones_rP[:], wb("rb2", 1),
                             start=False, stop=True)
        o_sb = const.tile([P, DOUT], f32, tag="osb")
        nc.vector.tensor_copy(o_sb[:], o_ps[:])
        # row-split across queues: descriptor-per-row DMA cost dominates
        for eng, r0, r1 in ((nc.sync, 0, 43), (nc.scalar, 43, 86),
                            (nc.gpsimd, 86, 128)):
            eng.dma_start(out_d[r0:r1, :], o_sb[r0:r1, :])

    nc.compile()
    return nc


def _score_bound_small(x_enc, g, thresh=1.0):
    """Sampled bound on |scores|/sqrt(dk): decides whether the cheap
    polynomial softplus is accurate enough (|y| <~ 1 -> abs err < 6e-3)."""
    def gelu(v):
        import math
        from numpy import vectorize
        try:
            from scipy.special import erf as _erf
        except Exception:
            _erf = vectorize(math.erf)
        return v * 0.5 * (1.0 + _erf(v / np.sqrt(2.0)))

    xs = x_enc.reshape(-1, DX)[:: max(1, (B * N) // 2048)]
    K = (gelu(gelu(xs @ g["kW1"] + g["kb1"]) @ g["kW2"] + g["kb2"])
         @ g["kW3"] + g["kb3"])
    k_max = np.linalg.norm(K, axis=1).max()
    q_max = (np.linalg.norm(g["query_tokens"][0], axis=1).max()
             + ADAPT_SCALE * np.abs(g["B_dirs"]).sum(0).max() * np.sqrt(DK))
    bound = 1.5 * k_max * q_max / np.sqrt(DK)
    return bool(bound < thresh * 8.0)


def _prep_maps(inputs):
    f32 = np.float32
    x_enc = np.asarray(inputs["x_enc"], f32)
    u = np.asarray(inputs["u"], f32)
    mask = np.asarray(inputs["sensor_mask"]).astype(f32)
    sw = np.asarray(inputs["sensor_weights"], f32)

    g = {k: np.asarray(inputs[k], f32) for k in
         ("kW1", "kb1", "kW2", "kb2", "kW3", "kb3",
          "vW1", "vb1", "vW2", "vb2", "vW3", "vb3",
          "query_tokens", "B_dirs", "aW1", "ab1", "aW2", "ab2",
          "rW1", "rb1", "rW2", "rb2")}

    biases_zero = bool(all(np.all(g[k] == 0)
                           for k in ("kb1", "vb1", "kb2", "vb2")))
    vb3_zero = bool(np.all(g["vb3"] == 0))
    use_poly = _score_bound_small(x_enc, g)
    ab2_zero = bool(np.all(g["ab2"] == 0))
    rb2_zero = bool(np.all(g["rb2"] == 0))
    rb1_zero = bool(np.all(g["rb1"] == 0))

    # --- bf16 weight pack ---
    wbp = np.zeros((128, WB_W), f32)

    def put(name, arr, row0=0):
        c0, c1 = WB_COLS[name]
        r, c = arr.shape
        assert c == c1 - c0, name
        wbp[row0:row0 + r, c0:c1] = arr

    put("kW1", g["kW1"])                       # rows 0-63 (64-67 stay zero)
    put("vW1", g["vW1"])                       # rows 0-67
    put("kW2a", g["kW2"][0:128]); put("kW2b", g["kW2"][128:256])
    put("vW2a", g["vW2"][0:128]); put("vW2b", g["vW2"][128:256])
    put("kW3a", g["kW3"][0:128]); put("kW3b", g["kW3"][128:256])
    put("vW3a", g["vW3"][0:128]); put("vW3b", g["vW3"][128:256])
    put("rW2a", g["rW2"][0:128]); put("rW2b", g["rW2"][128:256])
    put("rW1", g["rW1"])                       # rows 0-63
    put("rb2", g["rb2"].reshape(1, DOUT))
    # B_dirs rows flattened onto partition 0 so each row can be a
    # 1-contraction matmul stationary (rank-1 Q-delta accumulation)
    put("BdTf", (g["B_dirs"] * ADAPT_SCALE).reshape(1, R * DK))

    # --- f32 pack A (per-feature biases; sw/mk filled per core) ---
    fa_shared = np.zeros((128, FA_W), f32)
    for nm in ("kb1", "vb1", "kb2", "vb2", "rb1"):
        c0, c1 = FA_COLS[nm]
        fa_shared[:, c0:c1] = g[nm].reshape(2, 128).T
    fa_shared[:, 138:266] = np.vstack([g["query_tokens"][0].T] * 2)
    fa_shared[:, 266:267] = np.concatenate([g["kb3"], g["kb3"]]).reshape(128, 1)
    fa_shared[:, 267:271] = g["ab2"].reshape(P, R)

    # --- f32 pack B ---
    fbp = np.zeros((64, FB_W), f32)

    def putb(name, arr, row0=0):
        c0, c1 = FB_COLS[name]
        r, c = arr.shape
        assert c == c1 - c0, name
        fbp[row0:row0 + r, c0:c1] = arr

    putb("qT", g["query_tokens"][0].T)
    perm = (np.arange(P * R).reshape(R, P) * 0 +
            np.arange(P)[None, :] * R + np.arange(R)[:, None]).reshape(-1)
    putb("aW2", g["aW2"][:, perm])
    putb("aW1", g["aW1"])
    putb("ab1", g["ab1"].reshape(AH, 1))
    putb("kb3", g["kb3"].reshape(DK, 1))
    putb("vb3", g["vb3"].reshape(DV, 1))
    putb("Bd", g["B_dirs"] * ADAPT_SCALE)      # rows 0-3
    putb("ab2", g["ab2"].reshape(1, P * R)[:, perm])

    wb16 = wbp.astype(BF16)
    use_fp8 = os.environ.get("AQH_FP8", "0") == "1"
    f8np = ml_dtypes.float8_e4m3
    w8p = np.zeros((128, 1024), np.float32)
    w8p[:, 0:512] = (g["kW2"].reshape(2, 128, H).transpose(1, 0, 2)
                     .reshape(128, 2 * H))
    w8p[:, 512:1024] = (g["vW2"].reshape(2, 128, H).transpose(1, 0, 2)
                        .reshape(128, 2 * H))
    w8 = w8p.astype(f8np)
    in_maps = []
    for b in range(B):
        x_fm = x_enc[b].reshape(128, NT, DX).transpose(2, 1, 0).reshape(DX, N)
        u_fm = u[b].reshape(128, NT, DU).transpose(2, 1, 0).reshape(DU, N)
        xu_fm = np.concatenate([x_fm, u_fm], axis=0)
        fap = fa_shared.copy()
        fap[:, 0:64] = sw[b].reshape(128, NT)
        fap[:, 64:128] = mask[b].reshape(128, NT)
        m = {
            "xu_fm": np.ascontiguousarray(xu_fm).astype(BF16),
            "wb16lo": np.ascontiguousarray(wb16[0:68, 0:512]),
            "wb16hi": np.ascontiguousarray(wb16[:, 512:]),
            "f32a": fap,
            "f32b": fbp,
        }
        if use_fp8:
            m["w8"] = w8
        in_maps.append(m)
    return in_maps, (biases_zero, vb3_zero, ab2_zero, rb2_zero, use_poly,
                     rb1_zero, use_fp8)


def run(inputs, trace=False):
    _install_ntff_hook_shim()
    from concourse.bass_utils import run_bass_kernel_spmd

    in_maps, flags = _prep_maps(inputs)
    key = ("nc",) + flags
    if key not in _CACHE:
        _CACHE[key] = _build(*flags)
    nc = _CACHE[key]

    try:
        res = run_bass_kernel_spmd(nc, in_maps, core_ids=list(range(B)),
                                   trace=trace)
    except Exception:
        # transient device wedges (NRT_EXEC_UNIT_UNRECOVERABLE) usually
        # clear on the next attempt
        import time
        time.sleep(5)
        res = run_bass_kernel_spmd(nc, in_maps, core_ids=list(range(B)),
                                   trace=trace)
    out = np.stack([res.results[b]["out"] for b in range(B)], axis=0)
    return out.astype(np.float32), res


def kernel(**inputs) -> np.ndarray:
    out, _ = run(inputs, trace=False)
    return out

